# revision 13
# baseline (speedup 1.0000x reference)
"""Data-parallel AxialAttentionNet forward on 8 trn2 NeuronCores.

Shards the batch (N=16) across 8 cores (2 images each); params replicated
on every core (eval-mode BN is a pure affine, so no cross-device work).
"""
import os

# Neuron's compiler auto-casts f32 matmuls to bf16 by default, which costs
# ~3.5e-2 relative error on this net's logits. Force full f32. The axon boot
# shim stashes compiler flags in libneuronxla.libncc.NEURON_CC_FLAGS (which
# shadows the env var), so append there; keep the env var as a fallback for
# non-axon environments.
if 'auto-cast' not in os.environ.get('NEURON_CC_FLAGS', ''):
    os.environ['NEURON_CC_FLAGS'] = (
        os.environ.get('NEURON_CC_FLAGS', '') + ' --auto-cast=none').strip()

import numpy as np
import jax
import jax.numpy as jnp
from jax import lax

try:
    import libneuronxla.libncc as _libncc
    if _libncc.NEURON_CC_FLAGS and not any(
            'auto-cast' in f for f in _libncc.NEURON_CC_FLAGS):
        _libncc.NEURON_CC_FLAGS.append('--auto-cast=none')
except Exception:
    pass

EPS = 1e-5
G = 8  # attention groups

N_CORES = 8


def _bn(x, p):
    inv = 1.0 / np.sqrt(1.0 + EPS)
    shp = (1, -1) + (1,) * (x.ndim - 2)
    return x * (p['w'] * inv).reshape(shp) + p['b'].reshape(shp)


def _conv1x1(x, w):  # w: (O, C)
    return jnp.einsum('oc,bchw->bohw', w, x)


def _axial(x, p, width_axis, stride):
    if width_axis:
        x = jnp.swapaxes(x, 2, 3)
    N, C, H, W = x.shape
    q = _bn(_conv1x1(x, p['wq']), p['bn_q'])
    k = _bn(_conv1x1(x, p['wk']), p['bn_k'])
    v = _bn(_conv1x1(x, p['wv']), p['bn_v'])
    out_planes = v.shape[1]
    gp = out_planes // G
    q = q.reshape(N, G, gp // 2, H, W)
    k = k.reshape(N, G, gp // 2, H, W)
    v = v.reshape(N, G, gp, H, W)
    qr = _bn(jnp.einsum('bgciw,ci->bgiw', q, p['q_rel']), p['bn_qr'])[:, :, :, None, :]
    kr = _bn(jnp.einsum('bgciw,ci->bgiw', k, p['k_rel']), p['bn_kr'])[:, :, None, :, :]
    qk = jnp.einsum('bgciw,bgcjw->bgijw', q, k)
    qk = _bn(qk.reshape(N, G, H * H, W), p['bn_qk']).reshape(N, G, H, H, W)
    sim = jax.nn.softmax(qk + qr + kr, axis=3)
    sv = jnp.einsum('bgijw,bgcjw->bgciw', sim, v).reshape(N, out_planes, H, W)
    sve = jnp.einsum('bgijw,cj->bgciw', sim, p['v_rel']).reshape(N, out_planes, H, W)
    out = _bn(sv, p['bn_sv']) + _bn(sve, p['bn_sve'])
    if width_axis:
        out = jnp.swapaxes(out, 2, 3)
    if stride > 1:
        out = lax.reduce_window(out, 0.0, lax.add, (1, 1, stride, stride), (1, 1, stride, stride), 'VALID') / float(stride * stride)
    return out


def _block(x, p, stride):
    out = jax.nn.relu(_bn(_conv1x1(x, p['conv_down']), p['bn1']))
    out = _axial(out, p['att_h'], False, 1)
    out = _axial(out, p['att_w'], True, stride)
    out = jax.nn.relu(out)
    out = _bn(_conv1x1(out, p['conv_up']), p['bn2'])
    ds = p['downsample']
    if stride == 1:
        identity = _bn(_conv1x1(x, ds['w']), ds['bn'])
    else:
        # x[:, :, ::s, ::s] then 1x1 conv == strided 1x1 conv; avoids an
        # XLA gather that crashes the neuron compiler.
        identity = lax.conv_general_dilated(
            x, ds['w'][:, :, None, None], (stride, stride), 'VALID',
            dimension_numbers=('NCHW', 'OIHW', 'NCHW'))
        identity = _bn(identity, ds['bn'])
    return jax.nn.relu(out + identity)


def _forward(x, params):
    p = params
    h = lax.conv_general_dilated(x, p['stem']['conv'], (2, 2), [(3, 3), (3, 3)],
                                 dimension_numbers=('NCHW', 'OIHW', 'NCHW'))
    h = jax.nn.relu(_bn(h, p['stem']['bn']))
    h = lax.reduce_window(h, -jnp.inf, lax.max, (1, 1, 3, 3), (1, 1, 2, 2),
                          [(0, 0), (0, 0), (1, 1), (1, 1)])
    h = _block(h, p['layer1'], 1)
    h = _block(h, p['layer2'], 2)
    h = _block(h, p['layer3'], 2)
    h = _block(h, p['layer4'], 2)
    h = jnp.mean(h, axis=(2, 3))
    return h @ p['fc']['w'].T + p['fc']['b']


_pforward = jax.pmap(_forward, axis_name='dp', in_axes=(0, 0))


def _stem(x, p):
    h = lax.conv_general_dilated(x, p['stem']['conv'], (2, 2), [(3, 3), (3, 3)],
                                 dimension_numbers=('NCHW', 'OIHW', 'NCHW'))
    h = jax.nn.relu(_bn(h, p['stem']['bn']))
    return lax.reduce_window(h, -jnp.inf, lax.max, (1, 1, 3, 3), (1, 1, 2, 2),
                             [(0, 0), (0, 0), (1, 1), (1, 1)])


def _head(h, p):
    h = jnp.mean(h, axis=(2, 3))
    return h @ p['fc']['w'].T + p['fc']['b']


_pstem = jax.pmap(_stem, in_axes=(0, 0))
_pb1 = jax.pmap(lambda h, p: _block(h, p['layer1'], 1), in_axes=(0, 0))
_pb2 = jax.pmap(lambda h, p: _block(h, p['layer2'], 2), in_axes=(0, 0))
_pb3 = jax.pmap(lambda h, p: _block(h, p['layer3'], 2), in_axes=(0, 0))
_pb4 = jax.pmap(lambda h, p: _block(h, p['layer4'], 2), in_axes=(0, 0))
_phead = jax.pmap(_head, in_axes=(0, 0))

_STAGES = (_pstem, _pb1, _pb2, _pb3, _pb4, _phead)

_param_cache = {}


def _replicated_params(params):
    key = id(params)
    hit = _param_cache.get(key)
    if hit is not None:
        return hit
    devs = jax.devices()[:N_CORES]
    flat, treedef = jax.tree_util.tree_flatten(params)
    repl = [jax.device_put_replicated(leaf, devs) for leaf in flat]
    prepl = jax.tree_util.tree_unflatten(treedef, repl)
    _param_cache.clear()
    _param_cache[key] = prepl
    return prepl


def kernel(x, params):
    x = np.asarray(x)
    n = x.shape[0]
    xs = x.reshape(N_CORES, n // N_CORES, *x.shape[1:])
    prepl = _replicated_params(params)
    out = _pforward(xs, prepl)
    out = np.asarray(out).reshape(n, -1).astype(np.float32)
    return out
